# revision 28
# baseline (speedup 1.0000x reference)
"""DualGCN Trainium2 kernel: 8-core SPMD via bass/Tile.

Strategy: nodes row-sharded across 8 cores. Graph normalization
(symmetric deg^-1/2) is folded into host-built per-edge weights, so each
GCNConv becomes
    agg[d] = sum_e w'_e * h[src_e] + bias,    h = x @ W
with self-loops as ordinary edges (w' = dinv[d]^2).

Per conv: row-sharded matmul on the TensorEngine (bf16), AllGather of h
across the 8 cores, then an ELL-style pass loop: dma_gather fetches
h[src] rows for 1024 edge-slots at a time (round-robin over 4 SWDGE
queues to parallelize Q7 descriptor generation) and a fused
scalar_tensor_tensor accumulates acc += w'_k * G_k. Destinations are
locally sorted by in-degree so pass k covers only a prefix of slots.

The small ADT/pro conv needs no gathers at all: the host pre-expands
w'_e * x_ADT[src_e] into a chunk-major column stream and the whole
aggregation + W_p3 transform happens as PSUM-accumulated matmuls.

All outputs are produced in sim-slot order and un-permuted on the host.
"""
import sys

if '/opt/trn_rl_repo' not in sys.path:
    sys.path.insert(0, '/opt/trn_rl_repo')

import numpy as np

N = 20000
NCORES = 8
NLOC = N // NCORES            # 2500
CH = 20                       # chunks of 128 per core
NLOC_PAD = CH * 128           # 2560
IN_C = 2000
KPAD = 2048
HID = 512
OUT = 256
Q = 100
QPAD = 128
SUBBLK = 8                    # gather granularity (1024 idxs = dma_gather max)
NQ = 4                        # SWDGE queues for gather descriptor gen


def _wrap_idx(a):
    """Linear int array (len % 16 == 0) -> [128, len/16] int16 SWDGE layout."""
    t = a.reshape(-1, 16).T.astype(np.int16)
    return np.ascontiguousarray(np.tile(t, (8, 1)))


def _graph_prep(edge_index, edge_weight):
    """Per-graph host-side index prep + folded normalization weights."""
    src = np.asarray(edge_index[0], dtype=np.int64)
    dst = np.asarray(edge_index[1], dtype=np.int64)
    w = np.asarray(edge_weight, dtype=np.float32)
    loop = np.arange(N, dtype=np.int64)
    # self-loops FIRST so the stable sort puts them at k=0 of every row
    src_all = np.concatenate([loop, src])
    dst_all = np.concatenate([loop, dst])
    w_all = np.concatenate([np.ones(N, np.float32), w])

    order = np.argsort(dst_all, kind='stable')
    src_s = src_all[order]
    w_s = w_all[order]
    deg_cnt = np.bincount(dst_all, minlength=N)
    row_start = np.zeros(N + 1, np.int64)
    np.cumsum(deg_cnt, out=row_start[1:])

    # weighted degree (incl self-loop weight 1) -> dinv, folded edge coeffs
    deg_w = np.bincount(dst_all, weights=w_all, minlength=N)
    dinv = np.where(deg_w > 0, deg_w ** -0.5, 0.0).astype(np.float32)
    wf_s = dinv[src_s] * w_s * dinv[dst_all[order]]

    perms, islots, dls = [], [], []
    for c in range(NCORES):
        dl = deg_cnt[c * NLOC:(c + 1) * NLOC]
        perm = np.argsort(-dl, kind='stable')
        islot = np.empty(NLOC, np.int64)
        islot[perm] = np.arange(NLOC)
        perms.append(perm)
        islots.append(islot)
        dls.append(dl)

    D = int(max(dl.max() for dl in dls))
    NK128 = []
    for k in range(D):
        nk = max(int((dl[perm] > k).sum()) for dl, perm in zip(dls, perms))
        if k == 0:
            nk = NLOC_PAD
        NK128.append(max(128, ((nk + 127) // 128) * 128))

    well, srcs = [], []
    for c in range(NCORES):
        g0 = c * NLOC
        dl, perm = dls[c], perms[c]
        wp = np.zeros((NLOC_PAD, D), np.float32)
        node_of_slot = np.concatenate([g0 + perm, np.full(60, -1, np.int64)])
        deg_of_slot = np.concatenate([dl[perm], np.zeros(60, np.int64)])
        s_l = []
        for k in range(D):
            nk = NK128[k]
            sg = np.zeros(nk, np.int64)
            valid = (deg_of_slot[:nk] > k)
            vs = np.nonzero(valid)[0]
            if len(vs):
                e = row_start[node_of_slot[vs]] + k
                sg[vs] = src_s[e]
                wp[vs, k] = wf_s[e]
            s_l.append(sg)
        well.append(np.ascontiguousarray(
            wp.reshape(CH, 128, D).transpose(1, 0, 2)))   # [128, CH, D]
        srcs.append(s_l)

    return dict(perms=perms, islots=islots, D=D, NK128=NK128,
                well=well, src=srcs)


def _kch(NK128):
    """Passes covering chunk ch (chunk-major block count per chunk)."""
    return [sum(1 for nk in NK128 if nk >= (ch + 1) * 128)
            for ch in range(CH)]


def _gi_fast(gp, core, islot_tab, skip_k0=0):
    """Wrapped gather indices, CHUNK-major block order: for each dst chunk,
    all its passes consecutively. Rows index a table whose per-core rows are
    ordered by islot_tab [NCORES, NLOC] (nat-local -> table slot). With
    skip_k0 the self-loop pass is omitted (handled by a direct init load)."""
    kch = _kch(gp['NK128'])
    blocks = []
    for ch in range(CH):
        for k in range(skip_k0, kch[ch]):
            s = gp['src'][core][k][ch * 128:(ch + 1) * 128]
            idx = (s // NLOC) * NLOC_PAD + islot_tab[s // NLOC, s % NLOC]
            blocks.append(_wrap_idx(idx))
    return np.concatenate(blocks, axis=1)


def preprocess(inputs):
    import ml_dtypes
    bf16 = ml_dtypes.bfloat16
    x = np.asarray(inputs['x_RNA'], np.float32)
    xadt = np.asarray(inputs['x_ADT'], np.float32)
    gs = _graph_prep(inputs['sim_edge_index'], inputs['sim_edge_weight'])
    gd = _graph_prep(inputs['dist_edge_index'], inputs['dist_edge_weight'])
    gc = _graph_prep(inputs['common_edge_index'], inputs['common_edge_weight'])

    islot_s = np.stack(gs['islots'])          # [NCORES, NLOC]
    islot_d = np.stack(gd['islots'])
    islot_c = np.stack(gc['islots'])

    def pad_w(wm, kp):
        w_ = np.zeros((kp, wm.shape[1]), np.float32)
        w_[:wm.shape[0]] = wm
        return w_

    w1 = pad_w(np.asarray(inputs['W_rna1'], np.float32), KPAD).astype(bf16)
    w2 = pad_w(np.asarray(inputs['W_rna2'], np.float32), KPAD).astype(bf16)
    wsim = np.asarray(inputs['W_sim'], np.float32).astype(bf16)
    wdist = np.asarray(inputs['W_dist'], np.float32).astype(bf16)
    wp3 = pad_w(np.asarray(inputs['W_p3'], np.float32), QPAD).astype(bf16)
    wf1 = np.asarray(inputs['W_f1'], np.float32).astype(bf16)
    wf2 = np.asarray(inputs['W_f2'], np.float32).astype(bf16)

    def brep(b):
        return np.ascontiguousarray(
            np.broadcast_to(np.asarray(b, np.float32), (128, len(b))))

    common = dict(
        w1=w1, w2=w2, wsim=wsim, wdist=wdist, wp3=wp3, wf1=wf1, wf2=wf2,
        b1=brep(inputs['b_rna1']), b2=brep(inputs['b_rna2']),
        bsim=brep(inputs['b_sim']), bdist=brep(inputs['b_dist']),
        bp3=brep(inputs['b_p3']), bf1=brep(inputs['b_f1']),
        bf2=brep(inputs['b_f2']),
        ident=np.eye(128, dtype=np.float32),
    )

    # pro stream: chunk-major pre-weighted expanded x_ADT columns
    NKc, Dc = gc['NK128'], gc['D']
    K_ch = [sum(1 for k in range(Dc) if NKc[k] >= (ch + 1) * 128)
            for ch in range(CH)]
    sumcols = sum(K_ch) * 128
    xadt_pad = np.zeros((N, QPAD), np.float32)
    xadt_pad[:, :Q] = xadt

    in_maps = []
    for c in range(NCORES):
        own = slice(c * NLOC, (c + 1) * NLOC)
        # x rows in sim-slot order, transposed [KPAD, NLOC_PAD]
        xtn = np.zeros((KPAD, NLOC_PAD), np.float32)
        xtn[:IN_C, :NLOC] = x[own][gs['perms'][c]].T
        m = dict(common)
        m['xtn'] = xtn.astype(bf16)
        for tag, gp in (('s', gs), ('d', gd)):
            m[f'well_{tag}'] = np.ascontiguousarray(
                gp['well'][c].reshape(128, -1))
        # conv1 gathers index the s-slot-ordered h1 tables; conv2-sim's
        # table (h2s) is also s-slot-ordered, so gi_s serves both convs
        m['gi_s'] = _gi_fast(gs, c, islot_s, skip_k0=1)
        m['gi_d1'] = _gi_fast(gd, c, islot_s)
        m['gi_d2'] = _gi_fast(gd, c, islot_d, skip_k0=1)

        # pro stream [QPAD, sumcols] bf16, chunk-major
        stream = np.zeros((QPAD, sumcols), np.float32)
        wellc = gc['well'][c]                      # [128, CH, Dc]
        col = 0
        for ch in range(CH):
            for k in range(K_ch[ch]):
                sl = gc['src'][c][k][ch * 128:(ch + 1) * 128]
                wv = wellc[:, ch, k]
                stream[:, col:col + 128] = (xadt_pad[sl] * wv[:, None]).T
                col += 128
        assert col == sumcols
        m['prostream'] = stream.astype(bf16)

        # realign gathers (rows into 2560-row slot-ordered bounces)
        nat = gs['perms'][c]                       # s-slot -> nat local
        gi_xd = np.zeros(NLOC_PAD, np.int64)
        gi_pro = np.zeros(NLOC_PAD, np.int64)
        gi_xd[:NLOC] = islot_d[c][nat]
        gi_pro[:NLOC] = islot_c[c][nat]
        m['gi_xd'] = _wrap_idx(gi_xd)
        m['gi_pro'] = _wrap_idx(gi_pro)
        in_maps.append(m)

    meta = dict(gs=dict(D=gs['D'], NK128=gs['NK128']),
                gd=dict(D=gd['D'], NK128=gd['NK128']),
                K_ch=K_ch, sumcols=sumcols,
                perms_s=gs['perms'])
    return in_maps, meta


# ---------------------------------------------------------------------------
# device program
# ---------------------------------------------------------------------------

def _build(meta):
    import concourse.bass as bass
    import concourse.bacc as bacc
    import concourse.mybir as mybir
    import concourse.tile as tile

    f32 = mybir.dt.float32
    f32r = mybir.dt.float32r
    bf16 = mybir.dt.bfloat16
    i16 = mybir.dt.int16
    GDT = bf16
    MUL = mybir.AluOpType.mult
    ADD = mybir.AluOpType.add
    AFT = mybir.ActivationFunctionType

    Ds, Dd = meta['gs']['D'], meta['gd']['D']
    NKs, NKd = meta['gs']['NK128'], meta['gd']['NK128']
    K_ch, sumcols = meta['K_ch'], meta['sumcols']
    NALL = NCORES * NLOC_PAD

    nc = bacc.Bacc("TRN2", target_bir_lowering=False, debug=False,
                   num_devices=NCORES, num_swdge_queues=NQ)

    def din(name, shape, dt=f32):
        return nc.dram_tensor(name, shape, dt, kind="ExternalInput")

    xtn_in = din('xtn', [KPAD, NLOC_PAD], bf16)
    w1_in = din('w1', [KPAD, HID], bf16)
    w2_in = din('w2', [KPAD, HID], bf16)
    wsim_in = din('wsim', [HID, OUT], bf16)
    wdist_in = din('wdist', [HID, OUT], bf16)
    wp3_in = din('wp3', [QPAD, OUT], bf16)
    wf1_in = din('wf1', [HID, OUT], bf16)
    wf2_in = din('wf2', [HID, OUT], bf16)
    bias_in = {nm: din(nm, [128, w]) for nm, w in
               (('b1', HID), ('b2', HID), ('bsim', OUT), ('bdist', OUT),
                ('bp3', OUT), ('bf1', OUT), ('bf2', OUT))}
    ident_in = din('ident', [128, 128])
    well_in = {'s': din('well_s', [128, CH * Ds]),
               'd': din('well_d', [128, CH * Dd])}
    gi_in = {
        's': din('gi_s', [128, (sum(NKs) - NLOC_PAD) // 16], i16),
        'd1': din('gi_d1', [128, sum(NKd) // 16], i16),
        'd2': din('gi_d2', [128, (sum(NKd) - NLOC_PAD) // 16], i16),
        'xd': din('gi_xd', [128, NLOC_PAD // 16], i16),
        'pro': din('gi_pro', [128, NLOC_PAD // 16], i16),
    }
    prostream_in = din('prostream', [QPAD, sumcols], bf16)

    outs = {name: nc.dram_tensor(name, [NLOC_PAD, OUT], f32,
                                 kind="ExternalOutput")
            for name in ['x_sim_out', 'x_dist_out', 'fused_out',
                         'fused_pro_out', 'pro_out']}

    with tile.TileContext(nc) as tc:
        with tc.tile_pool(name="persist", bufs=1) as pp, \
             tc.tile_pool(name="gidx", bufs=3) as gip, \
             tc.tile_pool(name="diag", bufs=16) as dgp, \
             tc.tile_pool(name="dram", bufs=1, space="DRAM") as dram:

            def dtile(shape, tag, shared=False, dt=None):
                return dram.tile(shape, dt or GDT, tag=tag, name=tag,
                                 addr_space="Shared" if shared else "Local")
            bnc_h1s = dtile([NLOC_PAD, HID], "bnc_h1s")
            bnc_h1d = dtile([NLOC_PAD, HID], "bnc_h1d")
            bnc_h2s = dtile([NLOC_PAD, OUT], "bnc_h2s")
            bnc_h2d = dtile([NLOC_PAD, OUT], "bnc_h2d")
            bnc_xd = dtile([NLOC_PAD, OUT], "bnc_xd", dt=f32)
            bnc_pro = dtile([NLOC_PAD, OUT], "bnc_pro", dt=f32)
            ag_h1s = dtile([NALL, HID], "ag_h1s", True)
            ag_h1d = dtile([NALL, HID], "ag_h1d", True)
            ag_h2s = dtile([NALL, OUT], "ag_h2s", True)
            ag_h2d = dtile([NALL, OUT], "ag_h2d", True)

            def allgather(bounce, ag):
                nc.gpsimd.collective_compute(
                    "AllGather", mybir.AluOpType.bypass,
                    replica_groups=[list(range(NCORES))],
                    ins=[bounce.opt()], outs=[ag.opt()])

            # ---- persistent small tiles ----
            ident = pp.tile([128, 128], f32, tag="ident", name="ident")
            nc.sync.dma_start(ident[:], ident_in[:])
            identb = pp.tile([128, 128], bf16, tag="identb", name="identb")
            nc.scalar.activation(identb[:], ident[:], AFT.Copy)
            bias = {}
            for nm, t in bias_in.items():
                w = t.shape[1]
                bias[nm] = pp.tile([128, w], f32, tag=f"bias_{nm}",
                                   name=f"bias_{nm}")
                nc.sync.dma_start(bias[nm][:], t[:])
            well = {}
            for tag, D in (('s', Ds), ('d', Dd)):
                well[tag] = pp.tile([128, CH * D], f32, tag=f"well_{tag}",
                                    name=f"well_{tag}")
                nc.sync.dma_start(well[tag][:], well_in[tag][:])

            # ---- ELL pass loop, CHUNK-major, hybrid PE/DVE accumulate --
            def run_passes(gp, dgp, psp, acc, ag, gi_sb, wellt, D, NK, F,
                           bias_t, pe_frac, init_tbl=None):
                """acc[p,ch,:] = bias + sum_k well[p,ch,k]*table[src_k[slot]].

                Per chunk the first ~pe_frac of passes accumulate on the
                TensorEngine as a PSUM-resident chain of diag(w) @ G_block
                matmuls; the rest accumulate on the vector engine (STT).
                The PSUM partial is folded into acc at the chunk's last
                block. bias rides in the first DVE STT (or the flush).
                """
                kch = [sum(1 for nk in NK if nk >= (ch + 1) * 128)
                       for ch in range(CH)]
                k0 = 0
                if init_tbl is not None:
                    # self-loop pass via direct slot-ordered load: acc_ch =
                    # w0*init + bias; gathers start at k=1
                    k0 = 1
                    for ch in range(CH):
                        nc.vector.scalar_tensor_tensor(
                            acc[:, ch, :], init_tbl[:, ch, :],
                            wellt[:, ch * D: ch * D + 1], bias_t[:],
                            MUL, ADD)
                blocks = [(ch, k) for ch in range(CH)
                          for k in range(k0, kch[ch])]
                kp = [min(int(kch[ch] * pe_frac + 0.5), kch[ch])
                      for ch in range(CH)]
                psum = {}
                ngroups = (len(blocks) + SUBBLK - 1) // SUBBLK
                for g in range(ngroups):
                    grp = blocks[g * SUBBLK:(g + 1) * SUBBLK]
                    nb = len(grp)
                    base = g * SUBBLK * 8
                    G = gp.tile([128, SUBBLK, F], GDT, tag="G", name="G")
                    nc.gpsimd.dma_gather(
                        G[:, :nb, :], ag[:], gi_sb[:, base: base + nb * 8],
                        nb * 128, nb * 128, F, queue_num=g % NQ)
                    for i, (ch, k) in enumerate(grp):
                        wsl = wellt[:, ch * D + k: ch * D + k + 1]
                        if k < kp[ch]:
                            diag = dgp.tile([128, 128], GDT, tag="diag",
                                            name="diag")
                            nc.vector.tensor_scalar_mul(
                                diag[:], identb[:], wsl)
                            if k == k0:
                                psum[ch] = psp.tile([128, F], f32,
                                                    tag="psacc", name="psacc")
                            nc.tensor.matmul(
                                psum[ch][:], diag[:], G[:, i, :],
                                start=(k == k0), stop=(k == kp[ch] - 1))
                        else:
                            first_dve = (k == kp[ch] and init_tbl is None)
                            nc.vector.scalar_tensor_tensor(
                                acc[:, ch, :], G[:, i, :], wsl,
                                bias_t[:] if first_dve
                                else acc[:, ch, :], MUL, ADD)
                        if k == kch[ch] - 1 and kp[ch] > k0:
                            nc.vector.scalar_tensor_tensor(
                                acc[:, ch, :], psum[ch][:], 1.0,
                                bias_t[:] if (kp[ch] == kch[ch]
                                              and init_tbl is None)
                                else acc[:, ch, :], MUL, ADD)

            def load_gi(name):
                t = gip.tile([128, gi_in[name].shape[1]], i16, tag="gi",
                             name=f"gi_{name}")
                nc.sync.dma_start(t[:], gi_in[name][:])
                return t

            def load_init(pool, bnc, F, tg, nbufs):
                t = pool.tile([128, CH, F], GDT, tag=tg, name=f"gin_{tg}",
                              bufs=nbufs)
                nc.sync.dma_start(
                    t[:], bnc[:].rearrange("(b p) f -> p b f", p=128))
                return t

            # ---- P1: conv1 matmuls (two W-resident bf16 passes; the sim
            # pass finishes first so its AllGather kicks early) ----
            with tc.tile_pool(name="w12", bufs=2) as wp, \
                 tc.tile_pool(name="xt", bufs=3) as xp, \
                 tc.tile_pool(name="h1o", bufs=5) as hp, \
                 tc.tile_pool(name="psA", bufs=8, space="PSUM") as psA:
                for w_in, bnc, ag in ((w1_in, bnc_h1s, ag_h1s),
                                      (w2_in, bnc_h1d, ag_h1d)):
                    wsb = wp.tile([128, 16, HID], bf16, tag="w12",
                                  name="wsb")
                    nc.scalar.dma_start(
                        wsb[:], w_in[:].rearrange("(t p) n -> p t n", p=128))
                    for mgs in ((0, 1), (2, 3), (4,)):
                        nmg = len(mgs)
                        hgrp = hp.tile([128, 4 * nmg, HID], GDT, tag="hgrp",
                                       name="hgrp")
                        pss = [psA.tile([128, HID], f32, tag='ps', name='ps')
                               for _ in range(4 * nmg)]
                        for k in range(16):
                            xt_t = xp.tile([128, 512 * nmg], bf16, tag="xt",
                                           name="xt_t")
                            nc.sync.dma_start(
                                xt_t[:],
                                xtn_in[k * 128:(k + 1) * 128,
                                       mgs[0] * 512:
                                       (mgs[-1] + 1) * 512])
                            for j in range(4 * nmg):
                                nc.tensor.matmul(
                                    pss[j][:],
                                    xt_t[:, j * 128:(j + 1) * 128],
                                    wsb[:, k, :],
                                    start=(k == 0), stop=(k == 15))
                        for j in range(4 * nmg):
                            nc.scalar.activation(hgrp[:, j, :], pss[j][:],
                                                 AFT.Copy)
                        nc.scalar.dma_start(
                            bnc[mgs[0] * 512:(mgs[-1] + 1) * 512, :]
                            .rearrange("(b p) f -> p b f", p=128), hgrp[:])
                    allgather(bnc, ag)

            # ---- pro: stream matmuls, PSUM-accumulated (no gathers) ----
            with tc.tile_pool(name="pstr", bufs=3) as pstr, \
                 tc.tile_pool(name="prot", bufs=3) as prot, \
                 tc.tile_pool(name="wsp", bufs=1) as wsp, \
                 tc.tile_pool(name="psP", bufs=2, space="PSUM") as psP:
                wp3_sb = wsp.tile([128, OUT], bf16, tag="wp3", name="wp3_sb")
                nc.sync.dma_start(wp3_sb[:], wp3_in[:])
                col = 0
                for ch in range(CH):
                    kch = K_ch[ch]
                    st = pstr.tile([128, kch * 128], bf16, tag="st",
                                   name="st")
                    nc.sync.dma_start(
                        st[:], prostream_in[:, col * 128:(col + kch) * 128])
                    col += kch
                    pso = psP.tile([128, OUT], f32, tag="psp", name="psp")
                    for k in range(kch):
                        nc.tensor.matmul(pso[:],
                                         st[:, k * 128:(k + 1) * 128],
                                         wp3_sb[:],
                                         start=(k == 0), stop=(k == kch - 1))
                    pt = prot.tile([128, OUT], f32, tag="pt", name="pt")
                    nc.vector.scalar_tensor_tensor(
                        pt[:], pso[:], 1.0, bias['bp3'][:], MUL, ADD)
                    nc.scalar.dma_start(
                        bnc_pro[ch * 128:(ch + 1) * 128, :], pt[:])

            # ---- conv1 passes + conv2 matmuls ----
            accH_cm = tc.tile_pool(name="accH", bufs=1)
            accH = accH_cm.__enter__()
            gph_cm = tc.tile_pool(name="gathH", bufs=11)
            gph = gph_cm.__enter__()
            psH_cm = tc.tile_pool(name="psH", bufs=2, space="PSUM")
            psH = psH_cm.__enter__()

            xs = accH.tile([128, CH, HID], f32, tag="accH", name="xs")
            gi_s = load_gi('s')
            gin1 = load_init(gph, bnc_h1s, HID, "GinH", 1)
            run_passes(gph, dgp, psH, xs, ag_h1s, gi_s, well['s'], Ds, NKs,
                       HID, bias['b1'], 0.6, init_tbl=gin1)
            for ch in range(CH):
                nc.scalar.activation(xs[:, ch, :], xs[:, ch, :], AFT.Relu)

            def conv2_mm(xsrc, wsb, bnc, psB, psC, trp, hp2):
                for m in range(CH):
                    blocks = []
                    for kb in range(4):
                        tp = psB.tile([128, 128], f32, tag="tp", name="tp")
                        nc.tensor.transpose(
                            tp[:], xsrc[:, m, kb * 128:(kb + 1) * 128],
                            ident[:])
                        xb = trp.tile([128, 128], bf16, tag="xsT", name="xsT")
                        nc.scalar.activation(xb[:], tp[:], AFT.Copy)
                        blocks.append(xb)
                    pso = psC.tile([128, OUT], f32, tag="pso", name="pso")
                    for kb in range(4):
                        nc.tensor.matmul(pso[:], blocks[kb][:],
                                         wsb[:, kb, :],
                                         start=(kb == 0), stop=(kb == 3))
                    h2t = hp2.tile([128, OUT], GDT, tag="h2t", name="h2t")
                    nc.scalar.activation(h2t[:], pso[:], AFT.Copy)
                    nc.sync.dma_start(bnc[m * 128:(m + 1) * 128, :], h2t[:])

            _cm_w2 = tc.tile_pool(name="w2nd", bufs=1)
            _cm_tr = tc.tile_pool(name="tr", bufs=4)
            _cm_psB = tc.tile_pool(name="psB", bufs=3, space="PSUM")
            _cm_psC = tc.tile_pool(name="psC", bufs=2, space="PSUM")
            wp2 = _cm_w2.__enter__()
            trp = _cm_tr.__enter__()
            psB = _cm_psB.__enter__()
            psC = _cm_psC.__enter__()

            wsim_sb = wp2.tile([128, 4, OUT], bf16, tag="wsim",
                               name="wsim_sb")
            wdist_sb = wp2.tile([128, 4, OUT], bf16, tag="wdist",
                                name="wdist_sb")
            nc.sync.dma_start(
                wsim_sb[:], wsim_in[:].rearrange("(t p) n -> p t n", p=128))
            nc.sync.dma_start(
                wdist_sb[:],
                wdist_in[:].rearrange("(t p) n -> p t n", p=128))

            # conv2-sim mm -> bounce ; kick AG2s
            conv2_mm(xs, wsim_sb, bnc_h2s, psB, psC, trp, wp2)
            allgather(bnc_h2s, ag_h2s)

            # conv1-dist passes -> xd
            xd = accH.tile([128, CH, HID], f32, tag="accH", name="xd")
            gi = load_gi('d1')
            run_passes(gph, dgp, psH, xd, ag_h1d, gi, well['d'], Dd, NKd,
                       HID, bias['b2'], 0.6)
            for ch in range(CH):
                nc.scalar.activation(xd[:, ch, :], xd[:, ch, :], AFT.Relu)

            conv2_mm(xd, wdist_sb, bnc_h2d, psB, psC, trp, wp2)
            allgather(bnc_h2d, ag_h2d)

            _cm_psC.__exit__(None, None, None)
            _cm_psB.__exit__(None, None, None)
            _cm_tr.__exit__(None, None, None)
            _cm_w2.__exit__(None, None, None)
            psH_cm.__exit__(None, None, None)
            gph_cm.__exit__(None, None, None)
            accH_cm.__exit__(None, None, None)

            # ---- conv2 passes (OUT-wide accs) ----
            accO_cm = tc.tile_pool(name="accO", bufs=2)
            accO = accO_cm.__enter__()
            gpo_cm = tc.tile_pool(name="gathO", bufs=16)
            gpo = gpo_cm.__enter__()
            psO_cm = tc.tile_pool(name="psO", bufs=4, space="PSUM")
            psO = psO_cm.__enter__()

            acc2_s = accO.tile([128, CH, OUT], f32, tag="accO",
                               name="acc2_s")
            gin2 = load_init(gpo, bnc_h2s, OUT, "GinO", 2)
            run_passes(gpo, dgp, psO, acc2_s, ag_h2s, gi_s, well['s'], Ds,
                       NKs, OUT, bias['bsim'], 0.5, init_tbl=gin2)
            # x_sim is final here; write it out early
            nc.scalar.dma_start(
                outs['x_sim_out'][:].rearrange("(b p) f -> p b f", p=128),
                acc2_s[:])

            acc2_d = accO.tile([128, CH, OUT], f32, tag="accO",
                               name="acc2_d")
            gi = load_gi('d2')
            gin3 = load_init(gpo, bnc_h2d, OUT, "GinO", 2)
            run_passes(gpo, dgp, psO, acc2_d, ag_h2d, gi, well['d'], Dd,
                       NKd, OUT, bias['bdist'], 0.5, init_tbl=gin3)
            psO_cm.__exit__(None, None, None)
            gpo_cm.__exit__(None, None, None)

            # ---- realign x_dist & pro to sim-slot order ----
            with tc.tile_pool(name="ral", bufs=1) as ral:
                nc.scalar.dma_start(
                    bnc_xd[:].rearrange("(b p) f -> p b f", p=128),
                    acc2_d[:])
                gixd = load_gi('xd')
                gipro = load_gi('pro')
                xd_s = ral.tile([128, CH, OUT], f32, tag="xds", name="xd_s")
                pro_s = ral.tile([128, CH, OUT], f32, tag="pros",
                                 name="pro_s")
                for g0 in range(0, CH, SUBBLK):
                    nb = min(SUBBLK, CH - g0)
                    nc.gpsimd.dma_gather(
                        xd_s[:, g0:g0 + nb, :], bnc_xd[:],
                        gixd[:, g0 * 8:(g0 + nb) * 8],
                        nb * 128, nb * 128, OUT, queue_num=g0 // SUBBLK % NQ)
                    nc.gpsimd.dma_gather(
                        pro_s[:, g0:g0 + nb, :], bnc_pro[:],
                        gipro[:, g0 * 8:(g0 + nb) * 8],
                        nb * 128, nb * 128, OUT,
                        queue_num=(g0 // SUBBLK + 2) % NQ)

                # write outputs (sim-slot order; host unpermutes)
                nc.scalar.dma_start(
                    outs['x_dist_out'][:]
                    .rearrange("(b p) f -> p b f", p=128), xd_s[:])
                nc.scalar.dma_start(
                    outs['pro_out'][:].rearrange("(b p) f -> p b f", p=128),
                    pro_s[:])

                # ---- fused + fused_pro (operands SBUF-resident) ----
                with tc.tile_pool(name="fus", bufs=4) as fp, \
                     tc.tile_pool(name="wf", bufs=1) as wfp, \
                     tc.tile_pool(name="trf", bufs=6) as trf, \
                     tc.tile_pool(name="psF", bufs=4, space="PSUM") as psF, \
                     tc.tile_pool(name="psG", bufs=2, space="PSUM") as psG:
                    wf1_sb = wfp.tile([128, 4, OUT], bf16, tag="wf1",
                                      name="wf1_sb")
                    wf2_sb = wfp.tile([128, 4, OUT], bf16, tag="wf2",
                                      name="wf2_sb")
                    nc.sync.dma_start(
                        wf1_sb[:],
                        wf1_in[:].rearrange("(t p) n -> p t n", p=128))
                    nc.sync.dma_start(
                        wf2_sb[:],
                        wf2_in[:].rearrange("(t p) n -> p t n", p=128))

                    def tblocks(src_ap, n):
                        out = []
                        for kb in range(n):
                            tp = psF.tile([128, 128], f32, tag="tpf",
                                          name="tpf")
                            nc.tensor.transpose(
                                tp[:], src_ap[:, kb * 128:(kb + 1) * 128],
                                ident[:])
                            xb = trf.tile([128, 128], bf16, tag="fT",
                                          name="fT")
                            nc.scalar.activation(xb[:], tp[:], AFT.Copy)
                            out.append(xb)
                        return out

                    for m in range(CH):
                        r0, r1 = m * 128, (m + 1) * 128
                        blocks = (tblocks(acc2_s[:, m, :], 2)
                                  + tblocks(xd_s[:, m, :], 2))
                        psf = psG.tile([128, OUT], f32, tag="psf",
                                       name="psf")
                        for kb in range(4):
                            nc.tensor.matmul(psf[:], blocks[kb][:],
                                             wf1_sb[:, kb, :],
                                             start=(kb == 0), stop=(kb == 3))
                        fsd = fp.tile([128, OUT], f32, tag="fsd", name="fsd")
                        nc.vector.scalar_tensor_tensor(
                            fsd[:], psf[:], 1.0, bias['bf1'][:], MUL, ADD)
                        nc.sync.dma_start(outs['fused_out'][r0:r1, :],
                                          fsd[:])

                        blocks2 = tblocks(fsd[:], 2) + tblocks(
                            pro_s[:, m, :], 2)
                        psf2 = psG.tile([128, OUT], f32, tag="psf2",
                                        name="psf2")
                        for kb in range(4):
                            nc.tensor.matmul(psf2[:], blocks2[kb][:],
                                             wf2_sb[:, kb, :],
                                             start=(kb == 0), stop=(kb == 3))
                        fpd = fp.tile([128, OUT], f32, tag="fpd", name="fpd")
                        nc.vector.scalar_tensor_tensor(
                            fpd[:], psf2[:], 1.0, bias['bf2'][:], MUL, ADD)
                        nc.sync.dma_start(outs['fused_pro_out'][r0:r1, :],
                                          fpd[:])

            accO_cm.__exit__(None, None, None)

    nc.compile()
    return nc


_CACHE = {}


def kernel(**inputs):
    from concourse import bass_utils
    in_maps, meta = preprocess(inputs)
    key = (meta['gs']['D'], meta['gd']['D'],
           tuple(meta['gs']['NK128']), tuple(meta['gd']['NK128']),
           tuple(meta['K_ch']))
    if key not in _CACHE:
        _CACHE[key] = _build(meta)
    nc = _CACHE[key]
    res = bass_utils.run_bass_kernel_spmd(
        nc, in_maps, core_ids=list(range(NCORES)))
    global LAST_RESULTS
    LAST_RESULTS = res
    names = ['x_sim_out', 'x_dist_out', 'fused_out', 'fused_pro_out',
             'pro_out']
    full = []
    for n in names:
        parts = []
        for c in range(NCORES):
            slot = res.results[c][n][:NLOC]
            nat = np.empty_like(slot)
            nat[meta['perms_s'][c]] = slot
            parts.append(nat)
        full.append(np.concatenate(parts, axis=0))
    return tuple(full)


# revision 29
# speedup vs baseline: 1.0869x; 1.0869x over previous
"""DualGCN Trainium2 kernel: 8-core SPMD via bass/Tile.

Strategy: nodes row-sharded across 8 cores. Graph normalization
(symmetric deg^-1/2) is folded into host-built per-edge weights, so each
GCNConv becomes
    agg[d] = sum_e w'_e * h[src_e] + bias,    h = x @ W
with self-loops as ordinary edges (w' = dinv[d]^2).

Per conv: row-sharded matmul on the TensorEngine (bf16), AllGather of h
across the 8 cores, then an ELL-style pass loop: dma_gather fetches
h[src] rows for 1024 edge-slots at a time (round-robin over 4 SWDGE
queues to parallelize Q7 descriptor generation) and a fused
scalar_tensor_tensor accumulates acc += w'_k * G_k. Destinations are
locally sorted by in-degree so pass k covers only a prefix of slots.

The small ADT/pro conv needs no gathers at all: the host pre-expands
w'_e * x_ADT[src_e] into a chunk-major column stream and the whole
aggregation + W_p3 transform happens as PSUM-accumulated matmuls.

All outputs are produced in sim-slot order and un-permuted on the host.
"""
import sys

if '/opt/trn_rl_repo' not in sys.path:
    sys.path.insert(0, '/opt/trn_rl_repo')

import numpy as np

N = 20000
NCORES = 8
NLOC = N // NCORES            # 2500
CH = 20                       # chunks of 128 per core
NLOC_PAD = CH * 128           # 2560
IN_C = 2000
KPAD = 2048
HID = 512
OUT = 256
Q = 100
QPAD = 128
SUBBLK = 8                    # gather granularity (1024 idxs = dma_gather max)
NQ = 4                        # SWDGE queues for gather descriptor gen


def _wrap_idx(a):
    """Linear int array (len % 16 == 0) -> [128, len/16] int16 SWDGE layout."""
    t = a.reshape(-1, 16).T.astype(np.int16)
    return np.ascontiguousarray(np.tile(t, (8, 1)))


def _graph_prep(edge_index, edge_weight):
    """Per-graph host-side index prep + folded normalization weights."""
    src = np.asarray(edge_index[0], dtype=np.int64)
    dst = np.asarray(edge_index[1], dtype=np.int64)
    w = np.asarray(edge_weight, dtype=np.float32)
    loop = np.arange(N, dtype=np.int64)
    # self-loops FIRST so the stable sort puts them at k=0 of every row
    src_all = np.concatenate([loop, src])
    dst_all = np.concatenate([loop, dst])
    w_all = np.concatenate([np.ones(N, np.float32), w])

    order = np.argsort(dst_all, kind='stable')
    src_s = src_all[order]
    w_s = w_all[order]
    deg_cnt = np.bincount(dst_all, minlength=N)
    row_start = np.zeros(N + 1, np.int64)
    np.cumsum(deg_cnt, out=row_start[1:])

    # weighted degree (incl self-loop weight 1) -> dinv, folded edge coeffs
    deg_w = np.bincount(dst_all, weights=w_all, minlength=N)
    dinv = np.where(deg_w > 0, deg_w ** -0.5, 0.0).astype(np.float32)
    wf_s = dinv[src_s] * w_s * dinv[dst_all[order]]

    perms, islots, dls = [], [], []
    for c in range(NCORES):
        dl = deg_cnt[c * NLOC:(c + 1) * NLOC]
        perm = np.argsort(-dl, kind='stable')
        islot = np.empty(NLOC, np.int64)
        islot[perm] = np.arange(NLOC)
        perms.append(perm)
        islots.append(islot)
        dls.append(dl)

    D = int(max(dl.max() for dl in dls))
    NK128 = []
    for k in range(D):
        nk = max(int((dl[perm] > k).sum()) for dl, perm in zip(dls, perms))
        if k == 0:
            nk = NLOC_PAD
        NK128.append(max(128, ((nk + 127) // 128) * 128))

    well, srcs = [], []
    for c in range(NCORES):
        g0 = c * NLOC
        dl, perm = dls[c], perms[c]
        wp = np.zeros((NLOC_PAD, D), np.float32)
        node_of_slot = np.concatenate([g0 + perm, np.full(60, -1, np.int64)])
        deg_of_slot = np.concatenate([dl[perm], np.zeros(60, np.int64)])
        s_l = []
        for k in range(D):
            nk = NK128[k]
            sg = np.zeros(nk, np.int64)
            valid = (deg_of_slot[:nk] > k)
            vs = np.nonzero(valid)[0]
            if len(vs):
                e = row_start[node_of_slot[vs]] + k
                sg[vs] = src_s[e]
                wp[vs, k] = wf_s[e]
            s_l.append(sg)
        well.append(np.ascontiguousarray(
            wp.reshape(CH, 128, D).transpose(1, 0, 2)))   # [128, CH, D]
        srcs.append(s_l)

    return dict(perms=perms, islots=islots, D=D, NK128=NK128,
                well=well, src=srcs)


def _kch(NK128):
    """Passes covering chunk ch (chunk-major block count per chunk)."""
    return [sum(1 for nk in NK128 if nk >= (ch + 1) * 128)
            for ch in range(CH)]


def _gi_fast(gp, core, islot_tab, skip_k0=0):
    """Wrapped gather indices, CHUNK-major block order: for each dst chunk,
    all its passes consecutively. Rows index a table whose per-core rows are
    ordered by islot_tab [NCORES, NLOC] (nat-local -> table slot). With
    skip_k0 the self-loop pass is omitted (handled by a direct init load)."""
    kch = _kch(gp['NK128'])
    blocks = []
    for ch in range(CH):
        for k in range(skip_k0, kch[ch]):
            s = gp['src'][core][k][ch * 128:(ch + 1) * 128]
            idx = (s // NLOC) * NLOC_PAD + islot_tab[s // NLOC, s % NLOC]
            blocks.append(_wrap_idx(idx))
    return np.concatenate(blocks, axis=1)


def preprocess(inputs):
    import ml_dtypes
    bf16 = ml_dtypes.bfloat16
    x = np.asarray(inputs['x_RNA'], np.float32)
    xadt = np.asarray(inputs['x_ADT'], np.float32)
    gs = _graph_prep(inputs['sim_edge_index'], inputs['sim_edge_weight'])
    gd = _graph_prep(inputs['dist_edge_index'], inputs['dist_edge_weight'])
    gc = _graph_prep(inputs['common_edge_index'], inputs['common_edge_weight'])

    islot_s = np.stack(gs['islots'])          # [NCORES, NLOC]
    islot_d = np.stack(gd['islots'])
    islot_c = np.stack(gc['islots'])

    def pad_w(wm, kp):
        w_ = np.zeros((kp, wm.shape[1]), np.float32)
        w_[:wm.shape[0]] = wm
        return w_

    w1 = pad_w(np.asarray(inputs['W_rna1'], np.float32), KPAD).astype(bf16)
    w2 = pad_w(np.asarray(inputs['W_rna2'], np.float32), KPAD).astype(bf16)
    wsim = np.asarray(inputs['W_sim'], np.float32).astype(bf16)
    wdist = np.asarray(inputs['W_dist'], np.float32).astype(bf16)
    wp3 = pad_w(np.asarray(inputs['W_p3'], np.float32), QPAD).astype(bf16)
    wf1 = np.asarray(inputs['W_f1'], np.float32).astype(bf16)
    wf2 = np.asarray(inputs['W_f2'], np.float32).astype(bf16)

    def brep(b):
        return np.ascontiguousarray(
            np.broadcast_to(np.asarray(b, np.float32), (128, len(b))))

    common = dict(
        w1=w1, w2=w2, wsim=wsim, wdist=wdist, wp3=wp3, wf1=wf1, wf2=wf2,
        b1=brep(inputs['b_rna1']), b2=brep(inputs['b_rna2']),
        bsim=brep(inputs['b_sim']), bdist=brep(inputs['b_dist']),
        bp3=brep(inputs['b_p3']), bf1=brep(inputs['b_f1']),
        bf2=brep(inputs['b_f2']),
        ident=np.eye(128, dtype=np.float32),
    )

    # pro stream: chunk-major pre-weighted expanded x_ADT columns
    NKc, Dc = gc['NK128'], gc['D']
    K_ch = [sum(1 for k in range(Dc) if NKc[k] >= (ch + 1) * 128)
            for ch in range(CH)]
    sumcols = sum(K_ch) * 128
    xadt_pad = np.zeros((N, QPAD), np.float32)
    xadt_pad[:, :Q] = xadt

    in_maps = []
    for c in range(NCORES):
        own = slice(c * NLOC, (c + 1) * NLOC)
        # x rows in sim-slot order, transposed [KPAD, NLOC_PAD]
        xtn = np.zeros((KPAD, NLOC_PAD), np.float32)
        xtn[:IN_C, :NLOC] = x[own][gs['perms'][c]].T
        m = dict(common)
        m['xtn'] = xtn.astype(bf16)
        for tag, gp in (('s', gs), ('d', gd)):
            m[f'well_{tag}'] = np.ascontiguousarray(
                gp['well'][c].reshape(128, -1))
        # conv1 gathers index the s-slot-ordered h1 tables; conv2-sim's
        # table (h2s) is also s-slot-ordered, so gi_s serves both convs
        m['gi_s'] = _gi_fast(gs, c, islot_s)
        m['gi_d1'] = _gi_fast(gd, c, islot_s)
        m['gi_d2'] = _gi_fast(gd, c, islot_d)

        # pro stream [QPAD, sumcols] bf16, chunk-major
        stream = np.zeros((QPAD, sumcols), np.float32)
        wellc = gc['well'][c]                      # [128, CH, Dc]
        col = 0
        for ch in range(CH):
            for k in range(K_ch[ch]):
                sl = gc['src'][c][k][ch * 128:(ch + 1) * 128]
                wv = wellc[:, ch, k]
                stream[:, col:col + 128] = (xadt_pad[sl] * wv[:, None]).T
                col += 128
        assert col == sumcols
        m['prostream'] = stream.astype(bf16)

        # realign gathers (rows into 2560-row slot-ordered bounces)
        nat = gs['perms'][c]                       # s-slot -> nat local
        gi_xd = np.zeros(NLOC_PAD, np.int64)
        gi_pro = np.zeros(NLOC_PAD, np.int64)
        gi_xd[:NLOC] = islot_d[c][nat]
        gi_pro[:NLOC] = islot_c[c][nat]
        m['gi_xd'] = _wrap_idx(gi_xd)
        m['gi_pro'] = _wrap_idx(gi_pro)
        in_maps.append(m)

    meta = dict(gs=dict(D=gs['D'], NK128=gs['NK128']),
                gd=dict(D=gd['D'], NK128=gd['NK128']),
                K_ch=K_ch, sumcols=sumcols,
                perms_s=gs['perms'])
    return in_maps, meta


# ---------------------------------------------------------------------------
# device program
# ---------------------------------------------------------------------------

def _build(meta):
    import concourse.bass as bass
    import concourse.bacc as bacc
    import concourse.mybir as mybir
    import concourse.tile as tile

    f32 = mybir.dt.float32
    f32r = mybir.dt.float32r
    bf16 = mybir.dt.bfloat16
    i16 = mybir.dt.int16
    GDT = bf16
    MUL = mybir.AluOpType.mult
    ADD = mybir.AluOpType.add
    AFT = mybir.ActivationFunctionType

    Ds, Dd = meta['gs']['D'], meta['gd']['D']
    NKs, NKd = meta['gs']['NK128'], meta['gd']['NK128']
    K_ch, sumcols = meta['K_ch'], meta['sumcols']
    NALL = NCORES * NLOC_PAD

    nc = bacc.Bacc("TRN2", target_bir_lowering=False, debug=False,
                   num_devices=NCORES, num_swdge_queues=NQ)

    def din(name, shape, dt=f32):
        return nc.dram_tensor(name, shape, dt, kind="ExternalInput")

    xtn_in = din('xtn', [KPAD, NLOC_PAD], bf16)
    w1_in = din('w1', [KPAD, HID], bf16)
    w2_in = din('w2', [KPAD, HID], bf16)
    wsim_in = din('wsim', [HID, OUT], bf16)
    wdist_in = din('wdist', [HID, OUT], bf16)
    wp3_in = din('wp3', [QPAD, OUT], bf16)
    wf1_in = din('wf1', [HID, OUT], bf16)
    wf2_in = din('wf2', [HID, OUT], bf16)
    bias_in = {nm: din(nm, [128, w]) for nm, w in
               (('b1', HID), ('b2', HID), ('bsim', OUT), ('bdist', OUT),
                ('bp3', OUT), ('bf1', OUT), ('bf2', OUT))}
    ident_in = din('ident', [128, 128])
    well_in = {'s': din('well_s', [128, CH * Ds]),
               'd': din('well_d', [128, CH * Dd])}
    gi_in = {
        's': din('gi_s', [128, sum(NKs) // 16], i16),
        'd1': din('gi_d1', [128, sum(NKd) // 16], i16),
        'd2': din('gi_d2', [128, sum(NKd) // 16], i16),
        'xd': din('gi_xd', [128, NLOC_PAD // 16], i16),
        'pro': din('gi_pro', [128, NLOC_PAD // 16], i16),
    }
    prostream_in = din('prostream', [QPAD, sumcols], bf16)

    outs = {name: nc.dram_tensor(name, [NLOC_PAD, OUT], f32,
                                 kind="ExternalOutput")
            for name in ['x_sim_out', 'x_dist_out', 'fused_out',
                         'fused_pro_out', 'pro_out']}

    with tile.TileContext(nc) as tc:
        with tc.tile_pool(name="persist", bufs=1) as pp, \
             tc.tile_pool(name="gidx", bufs=3) as gip, \
             tc.tile_pool(name="diag", bufs=16) as dgp, \
             tc.tile_pool(name="dram", bufs=1, space="DRAM") as dram:

            def dtile(shape, tag, shared=False, dt=None):
                return dram.tile(shape, dt or GDT, tag=tag, name=tag,
                                 addr_space="Shared" if shared else "Local")
            bnc_h1s = dtile([NLOC_PAD, HID], "bnc_h1s")
            bnc_h1d = dtile([NLOC_PAD, HID], "bnc_h1d")
            bnc_h2s = dtile([NLOC_PAD, OUT], "bnc_h2s")
            bnc_h2d = dtile([NLOC_PAD, OUT], "bnc_h2d")
            bnc_xd = dtile([NLOC_PAD, OUT], "bnc_xd", dt=f32)
            bnc_pro = dtile([NLOC_PAD, OUT], "bnc_pro", dt=f32)
            ag_h1s = dtile([NALL, HID], "ag_h1s", True)
            ag_h1d = dtile([NALL, HID], "ag_h1d", True)
            ag_h2s = dtile([NALL, OUT], "ag_h2s", True)
            ag_h2d = dtile([NALL, OUT], "ag_h2d", True)

            def allgather(bounce, ag):
                nc.gpsimd.collective_compute(
                    "AllGather", mybir.AluOpType.bypass,
                    replica_groups=[list(range(NCORES))],
                    ins=[bounce.opt()], outs=[ag.opt()])

            # ---- persistent small tiles ----
            ident = pp.tile([128, 128], f32, tag="ident", name="ident")
            nc.sync.dma_start(ident[:], ident_in[:])
            identb = pp.tile([128, 128], bf16, tag="identb", name="identb")
            nc.scalar.activation(identb[:], ident[:], AFT.Copy)
            bias = {}
            for nm, t in bias_in.items():
                w = t.shape[1]
                bias[nm] = pp.tile([128, w], f32, tag=f"bias_{nm}",
                                   name=f"bias_{nm}")
                nc.sync.dma_start(bias[nm][:], t[:])
            well = {}
            for tag, D in (('s', Ds), ('d', Dd)):
                well[tag] = pp.tile([128, CH * D], f32, tag=f"well_{tag}",
                                    name=f"well_{tag}")
                nc.sync.dma_start(well[tag][:], well_in[tag][:])

            # ---- ELL pass loop, CHUNK-major, hybrid PE/DVE accumulate --
            def run_passes(gp, dgp, psp, acc, ag, gi_sb, wellt, D, NK, F,
                           bias_t, pe_frac, init_tbl=None):
                """acc[p,ch,:] = bias + sum_k well[p,ch,k]*table[src_k[slot]].

                Per chunk the first ~pe_frac of passes accumulate on the
                TensorEngine as a PSUM-resident chain of diag(w) @ G_block
                matmuls; the rest accumulate on the vector engine (STT).
                The PSUM partial is folded into acc at the chunk's last
                block. bias rides in the first DVE STT (or the flush).
                """
                kch = [sum(1 for nk in NK if nk >= (ch + 1) * 128)
                       for ch in range(CH)]
                k0 = 0
                if init_tbl is not None:
                    # self-loop pass via direct slot-ordered load: acc_ch =
                    # w0*init + bias; gathers start at k=1
                    k0 = 1
                    for ch in range(CH):
                        nc.vector.scalar_tensor_tensor(
                            acc[:, ch, :], init_tbl[:, ch, :],
                            wellt[:, ch * D: ch * D + 1], bias_t[:],
                            MUL, ADD)
                blocks = [(ch, k) for ch in range(CH)
                          for k in range(k0, kch[ch])]
                kp = [min(int(kch[ch] * pe_frac + 0.5), kch[ch])
                      for ch in range(CH)]
                psum = {}
                ngroups = (len(blocks) + SUBBLK - 1) // SUBBLK
                for g in range(ngroups):
                    grp = blocks[g * SUBBLK:(g + 1) * SUBBLK]
                    nb = len(grp)
                    base = g * SUBBLK * 8
                    G = gp.tile([128, SUBBLK, F], GDT, tag="G", name="G")
                    nc.gpsimd.dma_gather(
                        G[:, :nb, :], ag[:], gi_sb[:, base: base + nb * 8],
                        nb * 128, nb * 128, F, queue_num=g % NQ)
                    for i, (ch, k) in enumerate(grp):
                        wsl = wellt[:, ch * D + k: ch * D + k + 1]
                        if k < kp[ch]:
                            diag = dgp.tile([128, 128], GDT, tag="diag",
                                            name="diag")
                            nc.vector.tensor_scalar_mul(
                                diag[:], identb[:], wsl)
                            if k == k0:
                                psum[ch] = psp.tile([128, F], f32,
                                                    tag="psacc", name="psacc")
                            nc.tensor.matmul(
                                psum[ch][:], diag[:], G[:, i, :],
                                start=(k == k0), stop=(k == kp[ch] - 1))
                        else:
                            first_dve = (k == kp[ch] and init_tbl is None)
                            nc.vector.scalar_tensor_tensor(
                                acc[:, ch, :], G[:, i, :], wsl,
                                bias_t[:] if first_dve
                                else acc[:, ch, :], MUL, ADD)
                        if k == kch[ch] - 1 and kp[ch] > k0:
                            nc.vector.scalar_tensor_tensor(
                                acc[:, ch, :], psum[ch][:], 1.0,
                                bias_t[:] if (kp[ch] == kch[ch]
                                              and init_tbl is None)
                                else acc[:, ch, :], MUL, ADD)

            def load_gi(name):
                t = gip.tile([128, gi_in[name].shape[1]], i16, tag="gi",
                             name=f"gi_{name}")
                nc.sync.dma_start(t[:], gi_in[name][:])
                return t

            def load_init(pool, bnc, F, tg, nbufs):
                t = pool.tile([128, CH, F], GDT, tag=tg, name=f"gin_{tg}",
                              bufs=nbufs)
                nc.sync.dma_start(
                    t[:], bnc[:].rearrange("(b p) f -> p b f", p=128))
                return t

            # ---- P1: conv1 matmuls (two W-resident bf16 passes; the sim
            # pass finishes first so its AllGather kicks early) ----
            with tc.tile_pool(name="w12", bufs=2) as wp, \
                 tc.tile_pool(name="xt", bufs=3) as xp, \
                 tc.tile_pool(name="h1o", bufs=5) as hp, \
                 tc.tile_pool(name="psA", bufs=8, space="PSUM") as psA:
                for w_in, bnc, ag in ((w1_in, bnc_h1s, ag_h1s),
                                      (w2_in, bnc_h1d, ag_h1d)):
                    wsb = wp.tile([128, 16, HID], bf16, tag="w12",
                                  name="wsb")
                    nc.scalar.dma_start(
                        wsb[:], w_in[:].rearrange("(t p) n -> p t n", p=128))
                    for mgs in ((0, 1), (2, 3), (4,)):
                        nmg = len(mgs)
                        hgrp = hp.tile([128, 4 * nmg, HID], GDT, tag="hgrp",
                                       name="hgrp")
                        pss = [psA.tile([128, HID], f32, tag='ps', name='ps')
                               for _ in range(4 * nmg)]
                        for k in range(16):
                            xt_t = xp.tile([128, 512 * nmg], bf16, tag="xt",
                                           name="xt_t")
                            nc.sync.dma_start(
                                xt_t[:],
                                xtn_in[k * 128:(k + 1) * 128,
                                       mgs[0] * 512:
                                       (mgs[-1] + 1) * 512])
                            for j in range(4 * nmg):
                                nc.tensor.matmul(
                                    pss[j][:],
                                    xt_t[:, j * 128:(j + 1) * 128],
                                    wsb[:, k, :],
                                    start=(k == 0), stop=(k == 15))
                        for j in range(4 * nmg):
                            nc.scalar.activation(hgrp[:, j, :], pss[j][:],
                                                 AFT.Copy)
                        nc.scalar.dma_start(
                            bnc[mgs[0] * 512:(mgs[-1] + 1) * 512, :]
                            .rearrange("(b p) f -> p b f", p=128), hgrp[:])
                    allgather(bnc, ag)

            # ---- pro: stream matmuls, PSUM-accumulated (no gathers) ----
            with tc.tile_pool(name="pstr", bufs=3) as pstr, \
                 tc.tile_pool(name="prot", bufs=3) as prot, \
                 tc.tile_pool(name="wsp", bufs=1) as wsp, \
                 tc.tile_pool(name="psP", bufs=2, space="PSUM") as psP:
                wp3_sb = wsp.tile([128, OUT], bf16, tag="wp3", name="wp3_sb")
                nc.sync.dma_start(wp3_sb[:], wp3_in[:])
                col = 0
                for ch in range(CH):
                    kch = K_ch[ch]
                    st = pstr.tile([128, kch * 128], bf16, tag="st",
                                   name="st")
                    nc.sync.dma_start(
                        st[:], prostream_in[:, col * 128:(col + kch) * 128])
                    col += kch
                    pso = psP.tile([128, OUT], f32, tag="psp", name="psp")
                    for k in range(kch):
                        nc.tensor.matmul(pso[:],
                                         st[:, k * 128:(k + 1) * 128],
                                         wp3_sb[:],
                                         start=(k == 0), stop=(k == kch - 1))
                    pt = prot.tile([128, OUT], f32, tag="pt", name="pt")
                    nc.vector.scalar_tensor_tensor(
                        pt[:], pso[:], 1.0, bias['bp3'][:], MUL, ADD)
                    nc.scalar.dma_start(
                        bnc_pro[ch * 128:(ch + 1) * 128, :], pt[:])

            # ---- conv1 passes + conv2 matmuls ----
            accH_cm = tc.tile_pool(name="accH", bufs=1)
            accH = accH_cm.__enter__()
            gph_cm = tc.tile_pool(name="gathH", bufs=12)
            gph = gph_cm.__enter__()
            psH_cm = tc.tile_pool(name="psH", bufs=2, space="PSUM")
            psH = psH_cm.__enter__()

            xs = accH.tile([128, CH, HID], f32, tag="accH", name="xs")
            gi_s = load_gi('s')
            run_passes(gph, dgp, psH, xs, ag_h1s, gi_s, well['s'], Ds, NKs,
                       HID, bias['b1'], 0.6)
            for ch in range(CH):
                nc.scalar.activation(xs[:, ch, :], xs[:, ch, :], AFT.Relu)

            def conv2_mm(xsrc, wsb, bnc, psB, psC, trp, hp2):
                for m in range(CH):
                    blocks = []
                    for kb in range(4):
                        tp = psB.tile([128, 128], f32, tag="tp", name="tp")
                        nc.tensor.transpose(
                            tp[:], xsrc[:, m, kb * 128:(kb + 1) * 128],
                            ident[:])
                        xb = trp.tile([128, 128], bf16, tag="xsT", name="xsT")
                        nc.scalar.activation(xb[:], tp[:], AFT.Copy)
                        blocks.append(xb)
                    pso = psC.tile([128, OUT], f32, tag="pso", name="pso")
                    for kb in range(4):
                        nc.tensor.matmul(pso[:], blocks[kb][:],
                                         wsb[:, kb, :],
                                         start=(kb == 0), stop=(kb == 3))
                    h2t = hp2.tile([128, OUT], GDT, tag="h2t", name="h2t")
                    nc.scalar.activation(h2t[:], pso[:], AFT.Copy)
                    nc.sync.dma_start(bnc[m * 128:(m + 1) * 128, :], h2t[:])

            _cm_w2 = tc.tile_pool(name="w2nd", bufs=1)
            _cm_tr = tc.tile_pool(name="tr", bufs=4)
            _cm_psB = tc.tile_pool(name="psB", bufs=3, space="PSUM")
            _cm_psC = tc.tile_pool(name="psC", bufs=2, space="PSUM")
            wp2 = _cm_w2.__enter__()
            trp = _cm_tr.__enter__()
            psB = _cm_psB.__enter__()
            psC = _cm_psC.__enter__()

            wsim_sb = wp2.tile([128, 4, OUT], bf16, tag="wsim",
                               name="wsim_sb")
            wdist_sb = wp2.tile([128, 4, OUT], bf16, tag="wdist",
                                name="wdist_sb")
            nc.sync.dma_start(
                wsim_sb[:], wsim_in[:].rearrange("(t p) n -> p t n", p=128))
            nc.sync.dma_start(
                wdist_sb[:],
                wdist_in[:].rearrange("(t p) n -> p t n", p=128))

            # conv2-sim mm -> bounce ; kick AG2s
            conv2_mm(xs, wsim_sb, bnc_h2s, psB, psC, trp, wp2)
            allgather(bnc_h2s, ag_h2s)

            # conv1-dist passes -> xd
            xd = accH.tile([128, CH, HID], f32, tag="accH", name="xd")
            gi = load_gi('d1')
            run_passes(gph, dgp, psH, xd, ag_h1d, gi, well['d'], Dd, NKd,
                       HID, bias['b2'], 0.6)
            for ch in range(CH):
                nc.scalar.activation(xd[:, ch, :], xd[:, ch, :], AFT.Relu)

            conv2_mm(xd, wdist_sb, bnc_h2d, psB, psC, trp, wp2)
            allgather(bnc_h2d, ag_h2d)

            _cm_psC.__exit__(None, None, None)
            _cm_psB.__exit__(None, None, None)
            _cm_tr.__exit__(None, None, None)
            _cm_w2.__exit__(None, None, None)
            psH_cm.__exit__(None, None, None)
            gph_cm.__exit__(None, None, None)
            accH_cm.__exit__(None, None, None)

            # ---- conv2 passes (OUT-wide accs) ----
            accO_cm = tc.tile_pool(name="accO", bufs=2)
            accO = accO_cm.__enter__()
            gpo_cm = tc.tile_pool(name="gathO", bufs=16)
            gpo = gpo_cm.__enter__()
            psO_cm = tc.tile_pool(name="psO", bufs=4, space="PSUM")
            psO = psO_cm.__enter__()

            acc2_s = accO.tile([128, CH, OUT], f32, tag="accO",
                               name="acc2_s")
            run_passes(gpo, dgp, psO, acc2_s, ag_h2s, gi_s, well['s'], Ds,
                       NKs, OUT, bias['bsim'], 0.5)
            # x_sim is final here; write it out early
            nc.scalar.dma_start(
                outs['x_sim_out'][:].rearrange("(b p) f -> p b f", p=128),
                acc2_s[:])

            acc2_d = accO.tile([128, CH, OUT], f32, tag="accO",
                               name="acc2_d")
            gi = load_gi('d2')
            run_passes(gpo, dgp, psO, acc2_d, ag_h2d, gi, well['d'], Dd,
                       NKd, OUT, bias['bdist'], 0.5)
            psO_cm.__exit__(None, None, None)
            gpo_cm.__exit__(None, None, None)

            # ---- realign x_dist & pro to sim-slot order ----
            with tc.tile_pool(name="ral", bufs=1) as ral:
                nc.scalar.dma_start(
                    bnc_xd[:].rearrange("(b p) f -> p b f", p=128),
                    acc2_d[:])
                gixd = load_gi('xd')
                gipro = load_gi('pro')
                xd_s = ral.tile([128, CH, OUT], f32, tag="xds", name="xd_s")
                pro_s = ral.tile([128, CH, OUT], f32, tag="pros",
                                 name="pro_s")
                for g0 in range(0, CH, SUBBLK):
                    nb = min(SUBBLK, CH - g0)
                    nc.gpsimd.dma_gather(
                        xd_s[:, g0:g0 + nb, :], bnc_xd[:],
                        gixd[:, g0 * 8:(g0 + nb) * 8],
                        nb * 128, nb * 128, OUT, queue_num=g0 // SUBBLK % NQ)
                    nc.gpsimd.dma_gather(
                        pro_s[:, g0:g0 + nb, :], bnc_pro[:],
                        gipro[:, g0 * 8:(g0 + nb) * 8],
                        nb * 128, nb * 128, OUT,
                        queue_num=(g0 // SUBBLK + 2) % NQ)

                # write outputs (sim-slot order; host unpermutes)
                nc.scalar.dma_start(
                    outs['x_dist_out'][:]
                    .rearrange("(b p) f -> p b f", p=128), xd_s[:])
                nc.scalar.dma_start(
                    outs['pro_out'][:].rearrange("(b p) f -> p b f", p=128),
                    pro_s[:])

                # ---- fused + fused_pro (operands SBUF-resident) ----
                with tc.tile_pool(name="fus", bufs=4) as fp, \
                     tc.tile_pool(name="wf", bufs=1) as wfp, \
                     tc.tile_pool(name="trf", bufs=6) as trf, \
                     tc.tile_pool(name="psF", bufs=4, space="PSUM") as psF, \
                     tc.tile_pool(name="psG", bufs=2, space="PSUM") as psG:
                    wf1_sb = wfp.tile([128, 4, OUT], bf16, tag="wf1",
                                      name="wf1_sb")
                    wf2_sb = wfp.tile([128, 4, OUT], bf16, tag="wf2",
                                      name="wf2_sb")
                    nc.sync.dma_start(
                        wf1_sb[:],
                        wf1_in[:].rearrange("(t p) n -> p t n", p=128))
                    nc.sync.dma_start(
                        wf2_sb[:],
                        wf2_in[:].rearrange("(t p) n -> p t n", p=128))

                    def tblocks(src_ap, n):
                        out = []
                        for kb in range(n):
                            tp = psF.tile([128, 128], f32, tag="tpf",
                                          name="tpf")
                            nc.tensor.transpose(
                                tp[:], src_ap[:, kb * 128:(kb + 1) * 128],
                                ident[:])
                            xb = trf.tile([128, 128], bf16, tag="fT",
                                          name="fT")
                            nc.scalar.activation(xb[:], tp[:], AFT.Copy)
                            out.append(xb)
                        return out

                    for m in range(CH):
                        r0, r1 = m * 128, (m + 1) * 128
                        blocks = (tblocks(acc2_s[:, m, :], 2)
                                  + tblocks(xd_s[:, m, :], 2))
                        psf = psG.tile([128, OUT], f32, tag="psf",
                                       name="psf")
                        for kb in range(4):
                            nc.tensor.matmul(psf[:], blocks[kb][:],
                                             wf1_sb[:, kb, :],
                                             start=(kb == 0), stop=(kb == 3))
                        fsd = fp.tile([128, OUT], f32, tag="fsd", name="fsd")
                        nc.vector.scalar_tensor_tensor(
                            fsd[:], psf[:], 1.0, bias['bf1'][:], MUL, ADD)
                        nc.sync.dma_start(outs['fused_out'][r0:r1, :],
                                          fsd[:])

                        blocks2 = tblocks(fsd[:], 2) + tblocks(
                            pro_s[:, m, :], 2)
                        psf2 = psG.tile([128, OUT], f32, tag="psf2",
                                        name="psf2")
                        for kb in range(4):
                            nc.tensor.matmul(psf2[:], blocks2[kb][:],
                                             wf2_sb[:, kb, :],
                                             start=(kb == 0), stop=(kb == 3))
                        fpd = fp.tile([128, OUT], f32, tag="fpd", name="fpd")
                        nc.vector.scalar_tensor_tensor(
                            fpd[:], psf2[:], 1.0, bias['bf2'][:], MUL, ADD)
                        nc.sync.dma_start(outs['fused_pro_out'][r0:r1, :],
                                          fpd[:])

            accO_cm.__exit__(None, None, None)

    nc.compile()
    return nc


_CACHE = {}


def kernel(**inputs):
    from concourse import bass_utils
    in_maps, meta = preprocess(inputs)
    key = (meta['gs']['D'], meta['gd']['D'],
           tuple(meta['gs']['NK128']), tuple(meta['gd']['NK128']),
           tuple(meta['K_ch']))
    if key not in _CACHE:
        _CACHE[key] = _build(meta)
    nc = _CACHE[key]
    res = bass_utils.run_bass_kernel_spmd(
        nc, in_maps, core_ids=list(range(NCORES)))
    global LAST_RESULTS
    LAST_RESULTS = res
    names = ['x_sim_out', 'x_dist_out', 'fused_out', 'fused_pro_out',
             'pro_out']
    full = []
    for n in names:
        parts = []
        for c in range(NCORES):
            slot = res.results[c][n][:NLOC]
            nat = np.empty_like(slot)
            nat[meta['perms_s'][c]] = slot
            parts.append(nat)
        full.append(np.concatenate(parts, axis=0))
    return tuple(full)
